# revision 43
# baseline (speedup 1.0000x reference)
"""Self-contained Trainium2 kernel for nn_Attention_22814866276679.

Multi-head attention (ViT-style, 197 tokens, 12 heads, dim 768) with a
relative-position bias table, batch 64. Data-parallel over batch across the
8 NeuronCores (8 images per core, no collectives).

Math notes (host prep moves all layout work off the device):
  - qkv = x @ w_qkv.T + concat(q_bias, 0, v_bias); q *= 1/8. The 1/8 scale is
    folded into the pre-transposed q weights; the q bias is added per
    partition during the PSUM->SBUF copy (ScalarE activation bias); the v
    bias commutes past the softmax (weights sum to 1) so its projected image
    joins the output-projection bias, applied in the projection's
    PSUM->SBUF copy (fused DVE multiply-add).
  - scores are computed TRANSPOSED ([keys, queries]) so the softmax reduce
    (over keys) lands on the matmul contraction axis; no PE transposes.
  - relative-position bias enters as exp(scores+bias) = exp(scores)*exp(bias)
    with exp(bias) precomputed host-side; the elementwise multiply runs on
    GpSimdE (tensor_tensor), keeping both the PE (no bias identity matmuls)
    and the busy ScalarE/VectorE out of it.
  - |scores + bias| <= ~3 for these inputs, so exp() is computed without the
    max-subtraction (mathematically identical softmax).
  - V carries an appended ones column: the attention@V matmul then emits the
    softmax denominators as a 65th output row for free.
  - The whole kernel is software-pipelined per image: attention pairs for
    image b interleave with the q/k projection for images b+2,b+3, the V
    projection for image b+2, and the output projection for tokens whose
    aoT completed — every engine (PE/ACT/DVE/GPSIMD) stays fed throughout
    instead of phase-by-phase bursts.
  - input DMAs are split across the two hardware DGE queues (sync + scalar)
    so the input ramp overlaps; output is written bf16 (halves the out
    traffic; final f32 cast on host).
"""

import os
import sys

for _p in ("/opt/trn_rl_repo", "/root/.axon_site/_ro/trn_rl_repo"):
    if os.path.isdir(_p) and _p not in sys.path:
        sys.path.insert(0, _p)

import ml_dtypes
import numpy as np

import concourse.bass as bass
import concourse.mybir as mybir
import concourse.tile as tile
from concourse import bacc, library_config

BF16 = mybir.dt.bfloat16
F32 = mybir.dt.float32

B, N, DIM, H, HD = 64, 197, 768, 12, 64
NCORES = 8
BL = B // NCORES          # 8 images per core
TOK = BL * N              # 1576 tokens per core
C = 6                     # contraction chunks of 128 (768 = 6*128, no pad row)
CP = C * 128              # 768
NQ = 394                  # qk-projection free chunk = 2 images (4 * 394 = 1576)
PRJ = 384                 # v / output-projection free chunk (2 * 384 = 768)
N2 = 2 * N                # paired scores free size (keys 0:128 | keys 128:197)
FT = 2 * DIM // 128       # 12 q/k feature tiles (0-5: q, 6-11: k)

MUL = mybir.AluOpType.mult

# exp(bias) multiply: which engine per head parity (gpsimd keeps DVE/ACT free)
EB_ON_GPSIMD = True


def build_module(debug_taps: bool = False) -> bass.Bass:
    nc = bacc.Bacc()
    xt_d = nc.declare_dram_parameter("xt", [CP, TOK], BF16, isOutput=False)
    wqk_d = nc.declare_dram_parameter("wqk", [CP, 2 * DIM], BF16, isOutput=False)
    wv_d = nc.declare_dram_parameter("wv", [CP, DIM], BF16, isOutput=False)
    wp_d = nc.declare_dram_parameter("wp", [CP, DIM], BF16, isOutput=False)
    eb_d = nc.declare_dram_parameter("bpair", [128, H, N2], BF16, isOutput=False)
    qb_d = nc.declare_dram_parameter("qbias", [128, C], F32, isOutput=False)
    id_d = nc.declare_dram_parameter("ident", [128, 128], BF16, isOutput=False)
    pb_d = nc.declare_dram_parameter("pbias", [128, DIM], F32, isOutput=False)
    out_d = nc.declare_dram_parameter("out", [TOK, DIM], BF16, isOutput=True)

    with tile.TileContext(nc) as tc:
        with (
            tc.tile_pool(name="persist", bufs=1) as persist,
            tc.tile_pool(name="sb_e", bufs=5) as sb_e,
            tc.tile_pool(name="sb_r", bufs=8) as sb_r,
            tc.tile_pool(name="sb_rb", bufs=8) as sb_rb,
            tc.tile_pool(name="sb_out", bufs=6) as sb_out,
        ):
            xt = persist.tile([128, C, TOK], BF16)
            wqk = persist.tile([128, C, 2 * DIM], BF16)
            wv = persist.tile([128, C, DIM], BF16)
            wp = persist.tile([128, C, DIM], BF16)
            eb = persist.tile([128, H, N2], BF16)
            qb = persist.tile([128, C], F32)
            pbias = persist.tile([128, DIM], F32)
            # f 0-5: qT, 6-11: kT; +64 zero tail columns let the second
            # scores matmul always run M=128 (keys q0+128 .. q0+256)
            qkT = persist.tile([128, FT, TOK + 64], BF16)
            vst = persist.tile([128, 2 * BL, H, HD + 1], BF16)
            aoT = persist.tile([128, C, TOK], BF16)  # 6 feature chunks
            ident = persist.tile([128, 128], BF16)
            scratch = persist.tile([128, 128], BF16)

            # the identity (bias-matmul operand) comes from DRAM: nothing
            # may depend on early gpsimd work — load_library blocks the
            # gpsimd engine ~17us while the library image DMAs in. The
            # warm-up dummies run on a locally-memset scratch tile so they
            # start the moment the PE preamble finishes.
            nc.vector.memset(scratch[:, :], 0.0)
            nc.vector.memset(qkT[:, :, TOK:TOK + 64], 0.0)
            # partition_broadcast lives in 'proxy'
            nc.gpsimd.load_library(library_config.proxy)

            # ---- input DMAs, split across the two hardware DGE queues
            # (sync + scalar). Each dma_start costs ~0.7us of issuing-engine
            # time, so scalar only fronts the urgent wv chunks; its
            # remaining issues drip out between its PSUM->SBUF copies.
            nc.sync.dma_start(ident[:], id_d[:])
            nc.sync.dma_start(qb[:], qb_d[:])
            # first chunks split to first-consumer granularity (image 0 /
            # first v-projection half) so the first matmuls start ~3us
            # earlier on the DMA ramp
            for c in range(C):
                nc.sync.dma_start(xt[:, c, 0:N], xt_d[c * 128:(c + 1) * 128, 0:N])
                nc.scalar.dma_start(
                    wv[:, c, 0:PRJ], wv_d[c * 128:(c + 1) * 128, 0:PRJ]
                )
            for c in range(C):
                nc.sync.dma_start(
                    xt[:, c, N:TOK // 2], xt_d[c * 128:(c + 1) * 128, N:TOK // 2]
                )
                nc.scalar.dma_start(
                    wv[:, c, PRJ:DIM], wv_d[c * 128:(c + 1) * 128, PRJ:DIM]
                )
            nc.sync.dma_start(eb[:], eb_d[:])
            for c in range(C):
                nc.sync.dma_start(
                    wqk[:, c, DIM:2 * DIM],
                    wqk_d[c * 128:(c + 1) * 128, DIM:2 * DIM],
                )
            for c in range(C):
                nc.sync.dma_start(
                    xt[:, c, TOK // 2:TOK], xt_d[c * 128:(c + 1) * 128, TOK // 2:TOK]
                )

            scalar_dma_defer = [
                (wqk[:, c, 0:DIM], wqk_d[c * 128:(c + 1) * 128, 0:DIM])
                for c in range(C)
            ] + [
                (wp[:, c, :], wp_d[c * 128:(c + 1) * 128, :]) for c in range(C)
            ] + [(pbias[:], pb_d[:])]

            def drip_dma():
                if scalar_dma_defer:
                    nc.scalar.dma_start(*scalar_dma_defer.pop(0))

            with (
                tc.tile_pool(name="ps_s", bufs=3, space="PSUM") as ps_s,
                tc.tile_pool(name="ps_g", bufs=5, space="PSUM") as ps_g,
            ):
                # HAM warm-up dummies (no input deps): ~3us of back-to-back
                # matmuls during the DMA ramp flips the PE clock gate to
                # 8/8, so the real matmuls start at 2.4 GHz.
                wps = ps_g.tile([128, 128], F32, tag="g", name="warm")
                for wi in range(28):
                    nc.tensor.matmul(
                        wps[:, :], lhsT=scratch[:, :], rhs=scratch[:, :],
                        start=True, stop=True,
                    )

                # ---------- work-unit emitters ----------
                v_done = {}

                def emit_v_group(bt, m, tok0, n):
                    ps = ps_g.tile([128, PRJ], F32, tag="g", name=f"v_{bt}_{n}")
                    for c in range(C):
                        nc.tensor.matmul(
                            ps[0:m, :],
                            lhsT=xt[:, c, tok0:tok0 + m],
                            rhs=wv[:, c, n * PRJ:(n + 1) * PRJ],
                            start=(c == 0),
                            stop=(c == C - 1),
                        )
                    nc.scalar.copy(
                        vst[0:m, bt, n * 6:(n + 1) * 6, 0:HD],
                        ps[0:m, :].rearrange("p (h d) -> p h d", d=HD),
                    )
                    drip_dma()
                    v_done[bt] = v_done.get(bt, 0) + 1
                    if v_done[bt] == 2:
                        nc.vector.memset(vst[:, bt, :, HD:HD + 1], 1.0)

                def emit_qk_group(f, n):
                    ps = ps_g.tile([128, NQ], F32, tag="g", name=f"qk_{f}_{n}")
                    for c in range(C):
                        nc.tensor.matmul(
                            ps[:, :],
                            lhsT=wqk[:, c, f * 128:(f + 1) * 128],
                            rhs=xt[:, c, n * NQ:(n + 1) * NQ],
                            start=(c == 0),
                            stop=(c == C - 1),
                        )
                    if f < FT // 2:
                        # q tiles: add the (pre-scaled) q bias per
                        # partition during the PSUM->SBUF copy
                        nc.scalar.activation(
                            qkT[:, f, n * NQ:(n + 1) * NQ], ps[:, :],
                            mybir.ActivationFunctionType.Identity,
                            bias=qb[:, f:f + 1],
                        )
                    else:
                        nc.scalar.copy(qkT[:, f, n * NQ:(n + 1) * NQ], ps[:, :])
                    drip_dma()

                ob_tiles = {}
                proj_n_done = {}

                def emit_proj_group(j, n):
                    tok0 = j * 128
                    m = min(128, TOK - tok0)
                    ps = ps_g.tile([128, PRJ], F32, tag="g", name=f"pp_{j}_{n}")
                    for c in range(C):
                        nc.tensor.matmul(
                            ps[0:m, :],
                            lhsT=aoT[:, c, tok0:tok0 + m],
                            rhs=wp[:, c, n * PRJ:(n + 1) * PRJ],
                            start=(c == 0),
                            stop=(c == C - 1),
                        )
                    proj_finish(j, n, ps)

                def proj_finish(j, n, ps):
                    tok0 = j * 128
                    m = min(128, TOK - tok0)
                    ob = ob_tiles[j]
                    nc.vector.scalar_tensor_tensor(
                        out=ob[0:m, n * PRJ:(n + 1) * PRJ], in0=ps[0:m, :],
                        scalar=1.0, in1=pbias[0:m, n * PRJ:(n + 1) * PRJ],
                        op0=MUL, op1=mybir.AluOpType.add,
                    )
                    proj_n_done[j] += 1
                    if proj_n_done[j] == 2:
                        # the very last tiles go out on the (idle) scalar
                        # DGE queue so the final drain overlaps
                        eng = nc.scalar if j >= 10 else nc.sync
                        eng.dma_start(out_d[tok0:tok0 + m, :], ob[0:m, :])

                # ---------- work schedules ----------
                # Fillers (qk / v / out-proj groups) are spread just-in-time
                # so EVERY image's pair loop — the last ones included — has
                # independent PE work covering its softmax chains. qk chunk
                # n covers images 2n,2n+1 (n=0 in the prologue); scores of
                # pair (b, hp) only read features f=hp and f=6+hp, so late
                # chunks can land inside their own image's pair loop.
                qk_sched = {}  # (b, hp) -> (f, n)
                for n in (1, 2):
                    for i, f in enumerate(range(FT)):
                        b = 2 * (n - 1) + i // 6
                        qk_sched.setdefault((b, i % 6), []).append((f, n))
                for f in (0, 6, 1, 7):  # n=3, needed by pairs (6,0),(6,1)
                    qk_sched.setdefault((5, 2 + (0, 6, 1, 7).index(f)), []).append((f, 3))
                for i, f in enumerate((2, 8, 3, 9, 4, 10, 5, 11)):  # n=3 late
                    qk_sched.setdefault((6, i // 2), []).append((f, 3))
                # v for image vb: 4 groups, one image (two for vb<=3) ahead
                v_sched = {}
                for vb in range(2, BL):
                    b = vb - 2 if vb <= 3 else (vb - 1 if vb <= 6 else 6)
                    for t in range(2):
                        m = 128 if t == 0 else N - 128
                        bt = vb * 2 + t
                        for nn in range(2):
                            v_sched.setdefault((b, 2 * t + nn), []).append(
                                (bt, m, vb * N + t * 128, nn)
                            )
                # out-projection pops: rationed so the queue (20 groups,
                # j0..j9) drains exactly across the run, leaving image 7
                # one pop per pair
                pop_sched = {
                    1: (4, 5), 2: (4, 5), 3: (4, 5), 4: (2, 3, 4, 5),
                    5: (0, 1), 6: (4, 5), 7: (0, 1, 2, 3, 4, 5),
                }

                proj_ready = []

                # ---------- prologue: v images 0,1 + qk chunk n=0 ----------
                for vb in range(2):
                    for t in range(2):
                        m = 128 if t == 0 else N - 128
                        for nn in range(2):
                            emit_v_group(vb * 2 + t, m, vb * N + t * 128, nn)
                for f in range(FT):
                    emit_qk_group(f, 0)

                # ---------- main software-pipelined image loop ----------
                # tail j-tiles (tokens completing with image 7): their n=0
                # projection matmuls interleave into image 7's pair loop
                # (feature chunk c is complete after pair hp=c normalizes)
                tail_js = [10, 11, 12]
                tail_ps = {}

                def emit_pair(b, hp, pop):
                    q0 = b * N
                    if True:
                        pair = (2 * hp, 2 * hp + 1)
                        ss, es = {}, {}
                        # scoresT = biasT + k @ q.T in one PSUM bank per
                        # head. Bias matmul first (start=True, full tile);
                        # the second scores matmul runs M=128 using keys
                        # q0+128 .. q0+256 (spills into next image / zero
                        # tail — rows 69:128 of that half are never
                        # consumed). Even/odd heads sit on complementary PE
                        # row groups, so adjacent emission lets their K=64
                        # matmuls run concurrently.
                        for h in pair:
                            ss[h] = ps_s.tile([128, N2], F32, tag="s", name=f"s_{b}_{h}")
                            nc.tensor.matmul(
                                ss[h][:, :], lhsT=ident[:, :], rhs=eb[:, h, :],
                                start=True, stop=False,
                            )
                        for h in pair:
                            po, fq, fk = (h % 2) * 64, h // 2, FT // 2 + h // 2
                            nc.tensor.matmul(
                                ss[h][0:128, N:N2],
                                lhsT=qkT[po:po + 64, fk, q0 + 128:q0 + 256],
                                rhs=qkT[po:po + 64, fq, q0:q0 + N],
                                start=False, stop=False,
                            )
                        for h in pair:
                            po, fq, fk = (h % 2) * 64, h // 2, FT // 2 + h // 2
                            nc.tensor.matmul(
                                ss[h][0:128, 0:N],
                                lhsT=qkT[po:po + 64, fk, q0:q0 + 128],
                                rhs=qkT[po:po + 64, fq, q0:q0 + N],
                                start=False, stop=True,
                            )
                        for h in pair:
                            es[h] = sb_e.tile([128, N2], BF16, tag="e", name=f"e_{b}_{h}")
                            nc.scalar.activation(
                                es[h][:, :], ss[h][:, :],
                                mybir.ActivationFunctionType.Exp,
                            )

                        # PE filler between the scores and AV matmuls: covers
                        # the exp + bias-multiply latency with independent
                        # projection work.
                        for item in qk_sched.get((b, hp), ()):
                            emit_qk_group(*item)
                        for item in v_sched.get((b, hp), ()):
                            emit_v_group(*item)
                        if pop and proj_ready:
                            emit_proj_group(*proj_ready.pop(0))

                        # out.T (64 rows) + softmax denominators (row 64);
                        # BOTH heads of the pair share one PSUM bank
                        # ([128, 2, 197] = 1576B), so the denominator
                        # copy / reciprocal / broadcast run once per pair.
                        op = ps_g.tile([128, 2, N], F32, tag="g", name=f"o_{b}_{hp}")
                        for i, h in enumerate(pair):
                            nc.tensor.matmul(
                                op[0:HD + 1, i, :], lhsT=vst[:, b * 2, h, :],
                                rhs=es[h][0:128, 0:N],
                                start=(i == 0), stop=False,
                            )
                            nc.tensor.matmul(
                                op[0:HD + 1, i, :],
                                lhsT=vst[0:69, b * 2 + 1, h, :],
                                rhs=es[h][0:69, N:N2],
                                start=False, stop=(i == 1),
                            )
                        # custom-DVE reciprocal misreads PSUM; stage the
                        # denom rows in SBUF first (cross-window copy ok)
                        rc = sb_r.tile([1, 2, N], F32, tag="rc")
                        nc.vector.tensor_copy(rc[0:1, :, :], op[64:65, :, :])
                        rr = sb_r.tile([1, 2, N], F32, tag="rr")
                        nc.vector.reciprocal_approx_fast(rr[0:1, :, :], rc[0:1, :, :])
                        rb = sb_rb.tile([64, 2, N], F32)
                        nc.gpsimd.partition_broadcast(rb[0:64, :, :], rr[0:1, :, :])
                        for i, h in enumerate(pair):
                            po, fq = (h % 2) * 64, h // 2
                            nc.vector.scalar_tensor_tensor(
                                out=aoT[po:po + 64, fq, q0:q0 + N],
                                in0=op[0:64, i, :], scalar=1.0,
                                in1=rb[0:64, i, :],
                                op0=MUL, op1=MUL,
                            )

                        # image 7: tail-projection chunk c=hp (just
                        # normalized) for the last token tiles — keeps the
                        # PE dense through the final softmax chains
                        if b == BL - 1:
                            for j in tail_js:
                                tok0 = j * 128
                                m = min(128, TOK - tok0)
                                nc.tensor.matmul(
                                    tail_ps[j][0:m, :],
                                    lhsT=aoT[:, hp, tok0:tok0 + m],
                                    rhs=wp[:, hp, 0:PRJ],
                                    start=(hp == 0),
                                    stop=(hp == H // 2 - 1),
                                )

                def queue_projs(b):
                    # queue proj tiles whose token range completed with
                    # this image (tail tiles are handled separately)
                    for j in range((TOK + 127) // 128):
                        tok0 = j * 128
                        m = min(128, TOK - tok0)
                        if (tok0 + m - 1) // N != b or j in tail_js:
                            continue
                        ob_tiles[j] = sb_out.tile(
                            [128, DIM], BF16, tag="ob", name=f"ob_{j}"
                        )
                        proj_n_done[j] = 0
                        proj_ready.append((j, 0))
                        proj_ready.append((j, 1))

                for b in range(BL):
                    if b == BL - 1:
                        for j in tail_js:
                            ob_tiles[j] = sb_out.tile(
                                [128, DIM], BF16, tag="ob", name=f"ob_{j}"
                            )
                            proj_n_done[j] = 0
                            tail_ps[j] = ps_g.tile(
                                [128, PRJ], F32, tag="g", name=f"tp_{j}"
                            )
                    for hp in range(H // 2):
                        emit_pair(b, hp, hp in pop_sched.get(b, ()))
                    queue_projs(b)
                # final drain: leftovers, tail n=0 epilogues, the n=1 halves
                while proj_ready:
                    emit_proj_group(*proj_ready.pop(0))
                for j in tail_js:
                    proj_finish(j, 0, tail_ps[j])
                for j in tail_js:
                    emit_proj_group(j, 1)

    nc.finalize()
    return nc


def prep_shared(w_qkv, q_bias, v_bias, rel_table, w_proj, b_proj, rel_index):
    """Host-side weight/bias layouts shared by all cores (bf16)."""
    bf = ml_dtypes.bfloat16
    scale = HD ** -0.5

    wqk = np.empty((CP, 2 * DIM), np.float32)
    wqk[:, 0:DIM] = w_qkv[0:DIM].T * scale
    wqk[:, DIM:2 * DIM] = w_qkv[DIM:2 * DIM].T

    wv = np.ascontiguousarray(w_qkv[2 * DIM:3 * DIM].T)
    wp = np.ascontiguousarray(w_proj.T)
    # softmax weights sum to 1, so the V bias adds a constant v_bias per
    # query; its projected image joins the output-projection bias, which is
    # applied (f32, partition-replicated) during the PSUM->SBUF copy
    pbias = np.broadcast_to(
        (b_proj + w_proj @ v_bias).astype(np.float32), (128, DIM)
    ).copy()

    # bias[q, k, h] -> key-major pair layout bp[key%128, h, (key<128 ? q : N+q)]
    bmat = rel_table[rel_index]          # [197(q), 197(k), 12]
    bp = np.zeros((128, H, N2), np.float32)
    bp[:, :, 0:N] = bmat[:, 0:128, :].transpose(1, 2, 0)
    bp[0:69, :, N:N2] = bmat[:, 128:N, :].transpose(1, 2, 0)

    return {
        "wqk": wqk.astype(bf),
        "wv": wv.astype(bf),
        "wp": wp.astype(bf),
        "bpair": bp.astype(bf),
        "qbias": np.ascontiguousarray(
            (q_bias * scale).astype(np.float32).reshape(C, 128).T
        ),
        "ident": np.eye(128, dtype=np.float32).astype(bf),
        "pbias": pbias,
    }


def prep_core_x(x, core):
    bf = ml_dtypes.bfloat16
    xs = x[core * BL:(core + 1) * BL].reshape(TOK, DIM)
    return np.ascontiguousarray(xs.T).astype(bf)


_built = None


def kernel(**inputs) -> np.ndarray:
    global _built
    from concourse.bass_utils import run_bass_kernel_spmd

    x = np.asarray(inputs["x"], np.float32)
    shared = prep_shared(
        np.asarray(inputs["w_qkv"], np.float32),
        np.asarray(inputs["q_bias"], np.float32),
        np.asarray(inputs["v_bias"], np.float32),
        np.asarray(inputs["rel_table"], np.float32),
        np.asarray(inputs["w_proj"], np.float32),
        np.asarray(inputs["b_proj"], np.float32),
        np.asarray(inputs["rel_index"], np.int32),
    )
    in_maps = [dict(shared, xt=prep_core_x(x, i)) for i in range(NCORES)]

    if _built is None:
        _built = (None, build_module())
    res = run_bass_kernel_spmd(_built[1], in_maps, core_ids=list(range(NCORES)))
    out = np.concatenate(
        [
            np.asarray(res.results[i]["out"]).astype(np.float32).reshape(BL, N, DIM)
            for i in range(NCORES)
        ],
        axis=0,
    )
    return out


if __name__ == "__main__":
    nc = build_module()
    print("build OK")
